# revision 26
# baseline (speedup 1.0000x reference)
"""Binarized-weight MLP (BiMlp, 1w32a adaptive scaling) on 8 TRN2 NeuronCores.

Reference math (per token row x_t of [12544, 1024]):
    bw1 = mean(|w1|,axis=1,keepdims) * sign(w1)        # [4096, 1024]
    h   = gelu(x @ bw1.T + b1)                         # exact (erf) gelu
    bw2 = mean(|w2|,axis=1,keepdims) * sign(w2)        # [1024, 4096]
    out = h @ bw2.T + b2

Strategy: pure data-parallel over the 12544 tokens (1568/core, no collectives;
weights replicated — they fit in SBUF). All compute in a transposed layout
(hT = [H, tokens]) so per-output-channel alpha/bias ride the partition dim and
fuse into a single ScalarE activation (out = gelu(alpha*psum + bias)).
Matmuls run bf16 (sign weights are exact +-1 in bf16) accumulating fp32 in
PSUM; binarization (sign / |w| / alpha = mean|w|) is done on-device.

Host side only reshapes/transposes/shards (layout, not math) and reassembles.
"""

import os
import sys
import types

import numpy as np

N_CORES = 8
B, S, D, H = 64, 196, 1024, 4096
T_GLOBAL = B * S            # 12544 tokens
T = T_GLOBAL // N_CORES     # 1568 tokens per core
N_CHUNK = 4
TC = T // N_CHUNK           # 392 token columns per matmul (<=512 psum bank)
KD = D // 128               # 8 k-tiles over D
KH = H // 128               # 32 k-tiles over H


def _install_ntff_hook():
    """This container image lacks antenv.axon_hooks; synthesize it so
    run_bass_kernel_spmd(trace=True) can capture NTFF profiles through the
    libaxon_pjrt C ABI (the same hook trn_boot would register)."""
    if "antenv.axon_hooks" in sys.modules:
        return
    import contextlib
    import ctypes

    try:
        lib = ctypes.CDLL("/opt/axon/libaxon_pjrt.so")
        lib.axon_start_nrt_profile.argtypes = [
            ctypes.POINTER(ctypes.c_int64),
            ctypes.c_size_t,
        ]
        lib.axon_start_nrt_profile.restype = ctypes.c_int64
        lib.axon_stop_nrt_profile.argtypes = [ctypes.c_char_p]
        lib.axon_stop_nrt_profile.restype = ctypes.c_int64
    except (OSError, AttributeError):
        return

    @contextlib.contextmanager
    def _hook(output_dir, device_ids):
        import jax

        jax.devices()
        if device_ids:
            ids = (ctypes.c_int64 * len(device_ids))(*device_ids)
            rc = lib.axon_start_nrt_profile(ids, len(device_ids))
        else:
            rc = lib.axon_start_nrt_profile(None, 0)
        if rc != 0:
            raise RuntimeError(f"axon_start_nrt_profile rc={rc}")
        try:
            yield
        finally:
            n = lib.axon_stop_nrt_profile(str(output_dir).encode())
            print(f"ntff profile: {n} file(s) in {output_dir}", file=sys.stderr)

    mod = types.ModuleType("antenv.axon_hooks")
    mod.get_axon_ntff_profile_hook = lambda: _hook
    mod.set_axon_ntff_profile_hook = lambda h: None
    sys.modules["antenv.axon_hooks"] = mod


_install_ntff_hook()

import concourse.mybir as mybir  # noqa: E402
from concourse import bacc, tile  # noqa: E402
from concourse.bass import ts  # noqa: E402
from concourse.bass_utils import run_bass_kernel_spmd  # noqa: E402

F32 = mybir.dt.float32
BF16 = mybir.dt.bfloat16
AF = mybir.ActivationFunctionType
ALU = mybir.AluOpType


def build_kernel():
    nc = bacc.Bacc(
        "TRN2",
        target_bir_lowering=False,
        debug=False,
        enable_asserts=False,
        num_devices=N_CORES,
    )
    # x ships bf16 for the same reason as the weights: the bf16 matmul
    # consumes bf16(x) either way; host-side cast == the casting DMA it
    # replaces, but rides the fast HWDGE path at half the wire bytes.
    xt = nc.dram_tensor("xt", [D, T], BF16, kind="ExternalInput").ap()
    # weights ship as bf16: identical rounding to an on-device f32->bf16 cast,
    # half the wire bytes, and staging stays on the fast HWDGE (non-casting)
    # DMA path. sign/|.| of the bf16 value match sign/|.| of the f32 value
    # to bf16 precision, which is all the bf16 matmul consumes anyway.
    w1t = nc.dram_tensor("w1t", [D, H], BF16, kind="ExternalInput").ap()
    b1 = nc.dram_tensor("b1", [H], F32, kind="ExternalInput").ap()
    w2t = nc.dram_tensor("w2t", [H, D], BF16, kind="ExternalInput").ap()
    b2 = nc.dram_tensor("b2", [D], F32, kind="ExternalInput").ap()
    out = nc.dram_tensor("out", [D, T], F32, kind="ExternalOutput").ap()

    w1t_3d = w1t.rearrange("(k p) h -> k p h", p=128)   # [KD, 128, H]
    w2t_3d = w2t.rearrange("(k p) d -> k p d", p=128)   # [KH, 128, D]
    xt_3d = xt.rearrange("(k p) t -> p k t", p=128)     # [128, KD, T]
    out_3d = out.rearrange("(m p) t -> m p t", p=128)   # [KD, 128, T]

    with tile.TileContext(nc) as tc:
        with (
            tc.tile_pool(name="wb", bufs=1) as wbpool,
            tc.tile_pool(name="consts", bufs=1) as cpool,
        ):
            # persistent binarized weights (bf16 +-1)
            w1b = wbpool.tile([128, KD, H], BF16, tag="w1b")
            w2b = wbpool.tile([128, KH, D], BF16, tag="w2b")
            # per-output-channel constants, partition-major per 128-tile
            alpha1c = cpool.tile([128, KH], F32, tag="a1")
            alpha2c = cpool.tile([128, KD], F32, tag="a2")
            b1c = cpool.tile([128, KH], F32, tag="b1")
            b2c = cpool.tile([128, KD], F32, tag="b2")
            ones1 = cpool.tile([128, 1], BF16, tag="ones1")
            ones2 = cpool.tile([128, 1], F32, tag="ones2")

            nc.vector.memset(ones1[:], 1.0 / D)
            nc.vector.memset(ones2[:], 1.0 / H)
            nc.sync.dma_start(b1c[:], b1.rearrange("(m p) -> p m", p=128))
            nc.sync.dma_start(b2c[:], b2.rearrange("(m p) -> p m", p=128))

            dpool = tc.alloc_tile_pool(name="adram", bufs=1, space="DRAM")
            a1d = dpool.tile([H], F32, tag="a1d")
            a2d = dpool.tile([D], F32, tag="a2d")

            # ---- w1 prep ----
            # alpha row = (1/D * ones)^T @ |w1t| with ones STATIONARY: the
            # 1-column LDWEIGHTS is free and the N=512 moving operand keeps the
            # PE array dense (N=1 matmuls starve the HAM activity monitor and
            # re-throttle the clock to 1.2GHz). The [1, H] row accumulates
            # per-bank (each 512-chunk is exactly one PSUM bank), then bounces
            # through DRAM to become the per-partition column layout.
            with (
                tc.tile_pool(name="w1stage", bufs=4) as s1pool,
                tc.tile_pool(name="absw1", bufs=2) as a1pool,
                tc.tile_pool(name="row1", bufs=1) as r1pool,
                tc.tile_pool(name="psrow1", bufs=1, space="PSUM") as pr1,
            ):
                with nc.named_scope("w1prep"):
                    a1row_ps = pr1.tile([1, H], F32, tag="a1row")
                    # prime the HAM clock gate: ~20 dense throwaway matmuls
                    # while the weight DMA streams in, so the 2.4GHz clock is
                    # already up when real work lands (results overwritten by
                    # the k==0 start=True below).
                    # warm rhs is ZEROS so keepalive matmuls can accumulate
                    # +0.0 into the live alpha row without corrupting it
                    warm = cpool.tile([128, 512], BF16, tag="warm")
                    nc.vector.memset(warm[:], 0.0)
                    for _ in range(20):
                        nc.tensor.matmul(
                            a1row_ps[:, 0:512],
                            lhsT=ones1[:],
                            rhs=warm[:],
                            start=False,
                            stop=False,
                            skip_group_check=True,
                        )
                    for k in range(KD):
                        if k > 0:
                            # HAM-keepalive filler: no-dep matmuls the PE can
                            # chew on while waiting for the next weight tile,
                            # so the clock gate never sees an idle window.
                            for _ in range(6):
                                nc.tensor.matmul(
                                    a1row_ps[:, 0:512],
                                    lhsT=ones1[:],
                                    rhs=warm[:],
                                    start=False,
                                    stop=False,
                                    skip_group_check=True,
                                )
                        st = s1pool.tile([128, H], BF16, tag="w1stage")
                        nc.sync.dma_start(st[:], w1t_3d[k])
                        if k < 5:
                            nc.scalar.activation(w1b[:, k, :], st[:], AF.Sign)
                        else:
                            # DVE bit-trick sign to cut the serial ScalarE
                            # chain that gates fc1's first matmul
                            nc.vector.tensor_scalar(
                                w1b[:, k, :].bitcast(mybir.dt.uint16),
                                st[:].bitcast(mybir.dt.uint16),
                                0x8000,
                                0x3F80,
                                ALU.bitwise_and,
                                ALU.bitwise_or,
                            )
                        ab = a1pool.tile([128, H], BF16, tag="absw1")
                        nc.vector.tensor_scalar(
                            ab[:].bitcast(mybir.dt.uint16),
                            st[:].bitcast(mybir.dt.uint16),
                            0x7FFF,
                            None,
                            ALU.bitwise_and,
                        )
                        for n in range(H // 512):
                            nc.tensor.matmul(
                                a1row_ps[:, ts(n, 512)],
                                lhsT=ones1[:],
                                rhs=ab[:, ts(n, 512)],
                                start=(k == 0),
                                stop=(k == KD - 1),
                                skip_group_check=True,
                            )
                    a1row = r1pool.tile([1, H], F32, tag="a1row_sb")
                    nc.vector.tensor_copy(out=a1row[:], in_=a1row_ps[:])
                    nc.sync.dma_start(a1d[:], a1row[:])
                    nc.sync.dma_start(
                        alpha1c[:], a1d.rearrange("(m p) -> p m", p=128)
                    )

            # ---- main: 4 token chunks, fc1 -> gelu -> fc2 ----
            with (
                tc.tile_pool(name="absw2", bufs=2) as a2pool,
                tc.tile_pool(name="row2", bufs=1) as r2pool,
                tc.tile_pool(name="xc", bufs=2) as xpool,
                tc.tile_pool(name="ht", bufs=1) as hpool,
                tc.tile_pool(name="oc", bufs=3) as opool,
                tc.tile_pool(name="psrow2", bufs=1, space="PSUM") as pr2,
                tc.tile_pool(name="ps1", bufs=4, space="PSUM") as ps1pool,
                tc.tile_pool(name="ps2", bufs=2, space="PSUM") as ps2pool,
            ):
                # w2 lands directly in its persistent SBUF tile (no staging
                # slots to recycle): DMA raw -> abs to scratch -> accumulate
                # sum|w2| on DVE (f32) -> sign in place. All of it hides
                # under fc1_c0; the partition reduce is then a single fp32
                # ones-stationary matmul pair (~1us of PE instead of a 14us
                # N=512 burst).
                a2row_ps = pr2.tile([1, D], F32, tag="a2row")
                acc2 = r2pool.tile([128, D], F32, tag="acc2")

                for k in range(KH):
                    nc.sync.dma_start(w2b[:, k, :], w2t_3d[k])
                for k in range(KH):
                    ab2 = a2pool.tile([128, D], BF16, tag="absw2",
                                      name=f"ab2_{k}")
                    nc.vector.tensor_scalar(
                        ab2[:].bitcast(mybir.dt.uint16),
                        w2b[:, k, :].bitcast(mybir.dt.uint16),
                        0x7FFF,
                        None,
                        ALU.bitwise_and,
                    )
                    # sign on DVE (ScalarE is loaded with gelus + w1 signs):
                    # bf16 sign(w) == (w & 0x8000) | 0x3f80, in place, ordered
                    # after the abs read by same-engine program order.
                    nc.vector.tensor_scalar(
                        w2b[:, k, :].bitcast(mybir.dt.uint16),
                        w2b[:, k, :].bitcast(mybir.dt.uint16),
                        0x8000,
                        0x3F80,
                        ALU.bitwise_and,
                        ALU.bitwise_or,
                    )
                    if k == 0:
                        nc.vector.tensor_copy(out=acc2[:], in_=ab2[:])
                    else:
                        nc.vector.tensor_add(
                            out=acc2[:], in0=acc2[:], in1=ab2[:]
                        )

                for c in range(N_CHUNK):
                    csl = slice(c * TC, (c + 1) * TC)
                    with nc.named_scope(f"fc1_c{c}"):
                        xc = xpool.tile([128, KD, TC], BF16, tag="xc")
                        nc.sync.dma_start(xc[:], xt_3d[:, :, csl])
                        ht = hpool.tile([128, KH, TC], BF16, tag="ht")
                        for m in range(KH):
                            ps = ps1pool.tile([128, TC], F32, tag="ps1")
                            for k in range(KD):
                                nc.tensor.matmul(
                                    ps[:],
                                    lhsT=w1b[:, k, ts(m, 128)],
                                    rhs=xc[:, k, :],
                                    start=(k == 0),
                                    stop=(k == KD - 1),
                                )
                            nc.scalar.activation(
                                ht[:, m, :],
                                ps[:],
                                AF.Gelu,
                                bias=b1c[:, m : m + 1],
                                scale=alpha1c[:, m : m + 1],
                            )
                    if c == 0:
                        # alpha2 partition-reduce: one fp32 matmul pair over
                        # the DVE-accumulated sum|w2|, then the DRAM bounce
                        # into column layout.
                        with nc.named_scope("w2prep"):
                            for n in range(D // 512):
                                nc.tensor.matmul(
                                    a2row_ps[:, ts(n, 512)],
                                    lhsT=ones2[:],
                                    rhs=acc2[:, ts(n, 512)],
                                    start=True,
                                    stop=True,
                                    skip_group_check=True,
                                )
                            a2row = r2pool.tile([1, D], F32, tag="a2row_sb")
                            nc.vector.tensor_copy(out=a2row[:], in_=a2row_ps[:])
                            nc.sync.dma_start(a2d[:], a2row[:])
                            nc.sync.dma_start(
                                alpha2c[:], a2d.rearrange("(m p) -> p m", p=128)
                            )
                    with nc.named_scope(f"fc2_c{c}"):
                        for md in range(KD):
                            ps2 = ps2pool.tile([128, TC], F32, tag="ps2")
                            for mh in range(KH):
                                nc.tensor.matmul(
                                    ps2[:],
                                    lhsT=w2b[:, mh, ts(md, 128)],
                                    rhs=ht[:, mh, :],
                                    start=(mh == 0),
                                    stop=(mh == KH - 1),
                                )
                            oc = opool.tile([128, TC], F32, tag="oc")
                            nc.scalar.activation(
                                oc[:],
                                ps2[:],
                                AF.Identity,
                                bias=b2c[:, md : md + 1],
                                scale=alpha2c[:, md : md + 1],
                            )
                            nc.sync.dma_start(out_3d[md][:, csl], oc[:])

    nc.compile()
    return nc


_NC_CACHE = None


def _get_nc():
    global _NC_CACHE
    if _NC_CACHE is None:
        _NC_CACHE = build_kernel()
    return _NC_CACHE


def kernel(x, w1, b1, w2, b2):
    assert x.shape == (B, S, D) and w1.shape == (H, D) and w2.shape == (D, H)
    nc = _get_nc()

    import ml_dtypes

    xt = np.ascontiguousarray(x.reshape(T_GLOBAL, D).T).astype(
        ml_dtypes.bfloat16
    )                                                         # [D, 12544]
    w1t = np.ascontiguousarray(w1.T).astype(ml_dtypes.bfloat16)   # [D, H]
    w2t = np.ascontiguousarray(w2.T).astype(ml_dtypes.bfloat16)   # [H, D]
    b1 = np.ascontiguousarray(b1, dtype=np.float32)
    b2 = np.ascontiguousarray(b2, dtype=np.float32)

    in_maps = [
        {
            "xt": np.ascontiguousarray(xt[:, i * T : (i + 1) * T]),
            "w1t": w1t,
            "b1": b1,
            "w2t": w2t,
            "b2": b2,
        }
        for i in range(N_CORES)
    ]

    trace = bool(int(os.environ.get("BIMLP_TRACE", "0")))
    res = run_bass_kernel_spmd(
        nc, in_maps, core_ids=list(range(N_CORES)), trace=trace
    )
    if trace:
        kernel.last_results = res

    outt = np.concatenate([res.results[i]["out"] for i in range(N_CORES)], axis=1)
    return np.ascontiguousarray(outt.T).reshape(B, S, D).astype(np.float32)


# revision 28
# speedup vs baseline: 1.0388x; 1.0388x over previous
"""Binarized-weight MLP (BiMlp, 1w32a adaptive scaling) on 8 TRN2 NeuronCores.

Reference math (per token row x_t of [12544, 1024]):
    bw1 = mean(|w1|,axis=1,keepdims) * sign(w1)        # [4096, 1024]
    h   = gelu(x @ bw1.T + b1)                         # exact (erf) gelu
    bw2 = mean(|w2|,axis=1,keepdims) * sign(w2)        # [1024, 4096]
    out = h @ bw2.T + b2

Strategy: pure data-parallel over the 12544 tokens (1568/core, no collectives;
weights replicated — they fit in SBUF). All compute in a transposed layout
(hT = [H, tokens]) so per-output-channel alpha/bias ride the partition dim and
fuse into a single ScalarE activation (out = gelu(alpha*psum + bias)).
Matmuls run bf16 (sign weights are exact +-1 in bf16) accumulating fp32 in
PSUM; binarization (sign / |w| / alpha = mean|w|) is done on-device.

Host side only reshapes/transposes/shards (layout, not math) and reassembles.
"""

import os
import sys
import types

import numpy as np

N_CORES = 8
B, S, D, H = 64, 196, 1024, 4096
T_GLOBAL = B * S            # 12544 tokens
T = T_GLOBAL // N_CORES     # 1568 tokens per core
N_CHUNK = 4
TC = T // N_CHUNK           # 392 token columns per matmul (<=512 psum bank)
KD = D // 128               # 8 k-tiles over D
KH = H // 128               # 32 k-tiles over H


def _install_ntff_hook():
    """This container image lacks antenv.axon_hooks; synthesize it so
    run_bass_kernel_spmd(trace=True) can capture NTFF profiles through the
    libaxon_pjrt C ABI (the same hook trn_boot would register)."""
    if "antenv.axon_hooks" in sys.modules:
        return
    import contextlib
    import ctypes

    try:
        lib = ctypes.CDLL("/opt/axon/libaxon_pjrt.so")
        lib.axon_start_nrt_profile.argtypes = [
            ctypes.POINTER(ctypes.c_int64),
            ctypes.c_size_t,
        ]
        lib.axon_start_nrt_profile.restype = ctypes.c_int64
        lib.axon_stop_nrt_profile.argtypes = [ctypes.c_char_p]
        lib.axon_stop_nrt_profile.restype = ctypes.c_int64
    except (OSError, AttributeError):
        return

    @contextlib.contextmanager
    def _hook(output_dir, device_ids):
        import jax

        jax.devices()
        if device_ids:
            ids = (ctypes.c_int64 * len(device_ids))(*device_ids)
            rc = lib.axon_start_nrt_profile(ids, len(device_ids))
        else:
            rc = lib.axon_start_nrt_profile(None, 0)
        if rc != 0:
            raise RuntimeError(f"axon_start_nrt_profile rc={rc}")
        try:
            yield
        finally:
            n = lib.axon_stop_nrt_profile(str(output_dir).encode())
            print(f"ntff profile: {n} file(s) in {output_dir}", file=sys.stderr)

    mod = types.ModuleType("antenv.axon_hooks")
    mod.get_axon_ntff_profile_hook = lambda: _hook
    mod.set_axon_ntff_profile_hook = lambda h: None
    sys.modules["antenv.axon_hooks"] = mod


_install_ntff_hook()

import concourse.mybir as mybir  # noqa: E402
from concourse import bacc, tile  # noqa: E402
from concourse.bass import ts  # noqa: E402
from concourse.bass_utils import run_bass_kernel_spmd  # noqa: E402

F32 = mybir.dt.float32
BF16 = mybir.dt.bfloat16
AF = mybir.ActivationFunctionType
ALU = mybir.AluOpType


def build_kernel():
    nc = bacc.Bacc(
        "TRN2",
        target_bir_lowering=False,
        debug=False,
        enable_asserts=False,
        num_devices=N_CORES,
    )
    # x ships bf16 for the same reason as the weights: the bf16 matmul
    # consumes bf16(x) either way; host-side cast == the casting DMA it
    # replaces, but rides the fast HWDGE path at half the wire bytes.
    xt = nc.dram_tensor("xt", [D, T], BF16, kind="ExternalInput").ap()
    # weights ship as bf16: identical rounding to an on-device f32->bf16 cast,
    # half the wire bytes, and staging stays on the fast HWDGE (non-casting)
    # DMA path. sign/|.| of the bf16 value match sign/|.| of the f32 value
    # to bf16 precision, which is all the bf16 matmul consumes anyway.
    w1t = nc.dram_tensor("w1t", [D, H], BF16, kind="ExternalInput").ap()
    b1 = nc.dram_tensor("b1", [H], F32, kind="ExternalInput").ap()
    w2t = nc.dram_tensor("w2t", [H, D], BF16, kind="ExternalInput").ap()
    b2 = nc.dram_tensor("b2", [D], F32, kind="ExternalInput").ap()
    out = nc.dram_tensor("out", [D, T], F32, kind="ExternalOutput").ap()

    w1t_3d = w1t.rearrange("(k p) h -> k p h", p=128)   # [KD, 128, H]
    w2t_3d = w2t.rearrange("(k p) d -> k p d", p=128)   # [KH, 128, D]
    xt_3d = xt.rearrange("(k p) t -> p k t", p=128)     # [128, KD, T]
    out_3d = out.rearrange("(m p) t -> m p t", p=128)   # [KD, 128, T]

    with tile.TileContext(nc) as tc:
        with (
            tc.tile_pool(name="wb", bufs=1) as wbpool,
            tc.tile_pool(name="consts", bufs=1) as cpool,
        ):
            # persistent binarized weights (bf16 +-1)
            w1b = wbpool.tile([128, KD, H], BF16, tag="w1b")
            w2b = wbpool.tile([128, KH, D], BF16, tag="w2b")
            # per-output-channel constants, partition-major per 128-tile
            alpha1c = cpool.tile([128, KH], F32, tag="a1")
            alpha2c = cpool.tile([128, KD], F32, tag="a2")
            b1c = cpool.tile([128, KH], F32, tag="b1")
            b2c = cpool.tile([128, KD], F32, tag="b2")
            ones1 = cpool.tile([128, 1], BF16, tag="ones1")
            ones2 = cpool.tile([128, 1], F32, tag="ones2")

            nc.vector.memset(ones1[:], 1.0 / D)
            nc.vector.memset(ones2[:], 1.0 / H)
            nc.sync.dma_start(b1c[:], b1.rearrange("(m p) -> p m", p=128))
            nc.sync.dma_start(b2c[:], b2.rearrange("(m p) -> p m", p=128))

            dpool = tc.alloc_tile_pool(name="adram", bufs=1, space="DRAM")
            a1d = dpool.tile([H], F32, tag="a1d")
            a2d = dpool.tile([D], F32, tag="a2d")

            # ---- w1 prep ----
            # alpha row = (1/D * ones)^T @ |w1t| with ones STATIONARY: the
            # 1-column LDWEIGHTS is free and the N=512 moving operand keeps the
            # PE array dense (N=1 matmuls starve the HAM activity monitor and
            # re-throttle the clock to 1.2GHz). The [1, H] row accumulates
            # per-bank (each 512-chunk is exactly one PSUM bank), then bounces
            # through DRAM to become the per-partition column layout.
            with (
                tc.tile_pool(name="w1stage", bufs=4) as s1pool,
                tc.tile_pool(name="absw1", bufs=2) as a1pool,
                tc.tile_pool(name="row1", bufs=1) as r1pool,
                tc.tile_pool(name="psrow1", bufs=1, space="PSUM") as pr1,
            ):
                with nc.named_scope("w1prep"):
                    a1row_ps = pr1.tile([1, H], F32, tag="a1row")
                    # prime the HAM clock gate: ~20 dense throwaway matmuls
                    # while the weight DMA streams in, so the 2.4GHz clock is
                    # already up when real work lands (results overwritten by
                    # the k==0 start=True below).
                    # warm rhs is ZEROS so keepalive matmuls can accumulate
                    # +0.0 into the live alpha row without corrupting it
                    warm = cpool.tile([128, 512], BF16, tag="warm")
                    nc.vector.memset(warm[:], 0.0)
                    for _ in range(20):
                        nc.tensor.matmul(
                            a1row_ps[:, 0:512],
                            lhsT=ones1[:],
                            rhs=warm[:],
                            start=False,
                            stop=False,
                            skip_group_check=True,
                        )
                    for k in range(KD):
                        if k > 0:
                            # HAM-keepalive filler: no-dep matmuls the PE can
                            # chew on while waiting for the next weight tile,
                            # so the clock gate never sees an idle window.
                            for _ in range(6):
                                nc.tensor.matmul(
                                    a1row_ps[:, 0:512],
                                    lhsT=ones1[:],
                                    rhs=warm[:],
                                    start=False,
                                    stop=False,
                                    skip_group_check=True,
                                )
                        st = s1pool.tile([128, H], BF16, tag="w1stage")
                        nc.sync.dma_start(st[:], w1t_3d[k])
                        if k < 5:
                            nc.scalar.activation(w1b[:, k, :], st[:], AF.Sign)
                        else:
                            # DVE bit-trick sign to cut the serial ScalarE
                            # chain that gates fc1's first matmul
                            nc.vector.tensor_scalar(
                                w1b[:, k, :].bitcast(mybir.dt.uint16),
                                st[:].bitcast(mybir.dt.uint16),
                                0x8000,
                                0x3F80,
                                ALU.bitwise_and,
                                ALU.bitwise_or,
                            )
                        ab = a1pool.tile([128, H], BF16, tag="absw1")
                        nc.vector.tensor_scalar(
                            ab[:].bitcast(mybir.dt.uint16),
                            st[:].bitcast(mybir.dt.uint16),
                            0x7FFF,
                            None,
                            ALU.bitwise_and,
                        )
                        for n in range(H // 512):
                            nc.tensor.matmul(
                                a1row_ps[:, ts(n, 512)],
                                lhsT=ones1[:],
                                rhs=ab[:, ts(n, 512)],
                                start=(k == 0),
                                stop=(k == KD - 1),
                                skip_group_check=True,
                            )
                    a1row = r1pool.tile([1, H], F32, tag="a1row_sb")
                    nc.vector.tensor_copy(out=a1row[:], in_=a1row_ps[:])
                    nc.sync.dma_start(a1d[:], a1row[:])
                    nc.sync.dma_start(
                        alpha1c[:], a1d.rearrange("(m p) -> p m", p=128)
                    )

            # ---- main: 4 token chunks, fc1 -> gelu -> fc2 ----
            with (
                tc.tile_pool(name="absw2", bufs=2) as a2pool,
                tc.tile_pool(name="row2", bufs=1) as r2pool,
                tc.tile_pool(name="xc", bufs=2) as xpool,
                tc.tile_pool(name="ht", bufs=1) as hpool,
                tc.tile_pool(name="oc", bufs=3) as opool,
                tc.tile_pool(name="psrow2", bufs=1, space="PSUM") as pr2,
                tc.tile_pool(name="ps1", bufs=4, space="PSUM") as ps1pool,
                tc.tile_pool(name="ps2", bufs=2, space="PSUM") as ps2pool,
            ):
                # w2 lands directly in its persistent SBUF tile (no staging
                # slots to recycle): DMA raw -> abs to scratch -> accumulate
                # sum|w2| on DVE (f32) -> sign in place. All of it hides
                # under fc1_c0; the partition reduce is then a single fp32
                # ones-stationary matmul pair (~1us of PE instead of a 14us
                # N=512 burst).
                a2row_ps = pr2.tile([1, D], F32, tag="a2row")
                acc2 = r2pool.tile([128, D], F32, tag="acc2")

                # chunk-0 x lands BEFORE the bulk w2 stream: it is on fc1's
                # critical path and must not queue behind 8MB of weights.
                xc0 = xpool.tile([128, KD, TC], BF16, tag="xc", name="xc0")
                nc.sync.dma_start(xc0[:], xt_3d[:, :, 0:TC])

                for k in range(KH):
                    nc.sync.dma_start(w2b[:, k, :], w2t_3d[k])
                for k in range(KH):
                    ab2 = a2pool.tile([128, D], BF16, tag="absw2",
                                      name=f"ab2_{k}")
                    nc.vector.tensor_scalar(
                        ab2[:].bitcast(mybir.dt.uint16),
                        w2b[:, k, :].bitcast(mybir.dt.uint16),
                        0x7FFF,
                        None,
                        ALU.bitwise_and,
                    )
                    # sign on DVE (ScalarE is loaded with gelus + w1 signs):
                    # bf16 sign(w) == (w & 0x8000) | 0x3f80, in place, ordered
                    # after the abs read by same-engine program order.
                    nc.vector.tensor_scalar(
                        w2b[:, k, :].bitcast(mybir.dt.uint16),
                        w2b[:, k, :].bitcast(mybir.dt.uint16),
                        0x8000,
                        0x3F80,
                        ALU.bitwise_and,
                        ALU.bitwise_or,
                    )
                    if k == 0:
                        nc.vector.tensor_copy(out=acc2[:], in_=ab2[:])
                    else:
                        nc.vector.tensor_add(
                            out=acc2[:], in0=acc2[:], in1=ab2[:]
                        )

                for c in range(N_CHUNK):
                    csl = slice(c * TC, (c + 1) * TC)
                    with nc.named_scope(f"fc1_c{c}"):
                        if c == 0:
                            xc = xc0
                        else:
                            xc = xpool.tile([128, KD, TC], BF16, tag="xc")
                            nc.sync.dma_start(xc[:], xt_3d[:, :, csl])
                        ht = hpool.tile([128, KH, TC], BF16, tag="ht")
                        for m in range(KH):
                            ps = ps1pool.tile([128, TC], F32, tag="ps1")
                            for k in range(KD):
                                nc.tensor.matmul(
                                    ps[:],
                                    lhsT=w1b[:, k, ts(m, 128)],
                                    rhs=xc[:, k, :],
                                    start=(k == 0),
                                    stop=(k == KD - 1),
                                )
                            nc.scalar.activation(
                                ht[:, m, :],
                                ps[:],
                                AF.Gelu,
                                bias=b1c[:, m : m + 1],
                                scale=alpha1c[:, m : m + 1],
                            )
                    if c == 0:
                        # alpha2 partition-reduce: one fp32 matmul pair over
                        # the DVE-accumulated sum|w2|, then the DRAM bounce
                        # into column layout.
                        with nc.named_scope("w2prep"):
                            for n in range(D // 512):
                                nc.tensor.matmul(
                                    a2row_ps[:, ts(n, 512)],
                                    lhsT=ones2[:],
                                    rhs=acc2[:, ts(n, 512)],
                                    start=True,
                                    stop=True,
                                    skip_group_check=True,
                                )
                            a2row = r2pool.tile([1, D], F32, tag="a2row_sb")
                            nc.vector.tensor_copy(out=a2row[:], in_=a2row_ps[:])
                            nc.sync.dma_start(a2d[:], a2row[:])
                            nc.sync.dma_start(
                                alpha2c[:], a2d.rearrange("(m p) -> p m", p=128)
                            )
                    with nc.named_scope(f"fc2_c{c}"):
                        for md in range(KD):
                            ps2 = ps2pool.tile([128, TC], F32, tag="ps2")
                            for mh in range(KH):
                                nc.tensor.matmul(
                                    ps2[:],
                                    lhsT=w2b[:, mh, ts(md, 128)],
                                    rhs=ht[:, mh, :],
                                    start=(mh == 0),
                                    stop=(mh == KH - 1),
                                )
                            oc = opool.tile([128, TC], F32, tag="oc")
                            nc.scalar.activation(
                                oc[:],
                                ps2[:],
                                AF.Identity,
                                bias=b2c[:, md : md + 1],
                                scale=alpha2c[:, md : md + 1],
                            )
                            nc.sync.dma_start(out_3d[md][:, csl], oc[:])

    nc.compile()
    return nc


_NC_CACHE = None


def _get_nc():
    global _NC_CACHE
    if _NC_CACHE is None:
        _NC_CACHE = build_kernel()
    return _NC_CACHE


def kernel(x, w1, b1, w2, b2):
    assert x.shape == (B, S, D) and w1.shape == (H, D) and w2.shape == (D, H)
    nc = _get_nc()

    import ml_dtypes

    xt = np.ascontiguousarray(x.reshape(T_GLOBAL, D).T).astype(
        ml_dtypes.bfloat16
    )                                                         # [D, 12544]
    w1t = np.ascontiguousarray(w1.T).astype(ml_dtypes.bfloat16)   # [D, H]
    w2t = np.ascontiguousarray(w2.T).astype(ml_dtypes.bfloat16)   # [H, D]
    b1 = np.ascontiguousarray(b1, dtype=np.float32)
    b2 = np.ascontiguousarray(b2, dtype=np.float32)

    in_maps = [
        {
            "xt": np.ascontiguousarray(xt[:, i * T : (i + 1) * T]),
            "w1t": w1t,
            "b1": b1,
            "w2t": w2t,
            "b2": b2,
        }
        for i in range(N_CORES)
    ]

    trace = bool(int(os.environ.get("BIMLP_TRACE", "0")))
    res = run_bass_kernel_spmd(
        nc, in_maps, core_ids=list(range(N_CORES)), trace=trace
    )
    if trace:
        kernel.last_results = res

    outt = np.concatenate([res.results[i]["out"] for i in range(N_CORES)], axis=1)
    return np.ascontiguousarray(outt.T).reshape(B, S, D).astype(np.float32)


# revision 33
# speedup vs baseline: 1.0572x; 1.0177x over previous
"""Binarized-weight MLP (BiMlp, 1w32a adaptive scaling) on 8 TRN2 NeuronCores.

Reference math (per token row x_t of [12544, 1024]):
    bw1 = mean(|w1|,axis=1,keepdims) * sign(w1)        # [4096, 1024]
    h   = gelu(x @ bw1.T + b1)                         # exact (erf) gelu
    bw2 = mean(|w2|,axis=1,keepdims) * sign(w2)        # [1024, 4096]
    out = h @ bw2.T + b2

Strategy: pure data-parallel over the 12544 tokens (1568/core, no collectives;
weights replicated — they fit in SBUF). All compute in a transposed layout
(hT = [H, tokens]) so per-output-channel alpha/bias ride the partition dim and
fuse into a single ScalarE activation (out = gelu(alpha*psum + bias)).
Matmuls run bf16 (sign weights are exact +-1 in bf16) accumulating fp32 in
PSUM; binarization (sign / |w| / alpha = mean|w|) is done on-device.

Host side only reshapes/transposes/shards (layout, not math) and reassembles.
"""

import os
import sys
import types

import numpy as np

N_CORES = 8
B, S, D, H = 64, 196, 1024, 4096
T_GLOBAL = B * S            # 12544 tokens
T = T_GLOBAL // N_CORES     # 1568 tokens per core
N_CHUNK = 4
TC = T // N_CHUNK           # 392 token columns per matmul (<=512 psum bank)
KD = D // 128               # 8 k-tiles over D
KH = H // 128               # 32 k-tiles over H


def _install_ntff_hook():
    """This container image lacks antenv.axon_hooks; synthesize it so
    run_bass_kernel_spmd(trace=True) can capture NTFF profiles through the
    libaxon_pjrt C ABI (the same hook trn_boot would register)."""
    if "antenv.axon_hooks" in sys.modules:
        return
    import contextlib
    import ctypes

    try:
        lib = ctypes.CDLL("/opt/axon/libaxon_pjrt.so")
        lib.axon_start_nrt_profile.argtypes = [
            ctypes.POINTER(ctypes.c_int64),
            ctypes.c_size_t,
        ]
        lib.axon_start_nrt_profile.restype = ctypes.c_int64
        lib.axon_stop_nrt_profile.argtypes = [ctypes.c_char_p]
        lib.axon_stop_nrt_profile.restype = ctypes.c_int64
    except (OSError, AttributeError):
        return

    @contextlib.contextmanager
    def _hook(output_dir, device_ids):
        import jax

        jax.devices()
        if device_ids:
            ids = (ctypes.c_int64 * len(device_ids))(*device_ids)
            rc = lib.axon_start_nrt_profile(ids, len(device_ids))
        else:
            rc = lib.axon_start_nrt_profile(None, 0)
        if rc != 0:
            raise RuntimeError(f"axon_start_nrt_profile rc={rc}")
        try:
            yield
        finally:
            n = lib.axon_stop_nrt_profile(str(output_dir).encode())
            print(f"ntff profile: {n} file(s) in {output_dir}", file=sys.stderr)

    mod = types.ModuleType("antenv.axon_hooks")
    mod.get_axon_ntff_profile_hook = lambda: _hook
    mod.set_axon_ntff_profile_hook = lambda h: None
    sys.modules["antenv.axon_hooks"] = mod


_install_ntff_hook()

import concourse.mybir as mybir  # noqa: E402
from concourse import bacc, tile  # noqa: E402
from concourse.bass import ts  # noqa: E402
from concourse.bass_utils import run_bass_kernel_spmd  # noqa: E402

F32 = mybir.dt.float32
BF16 = mybir.dt.bfloat16
AF = mybir.ActivationFunctionType
ALU = mybir.AluOpType


def build_kernel():
    nc = bacc.Bacc(
        "TRN2",
        target_bir_lowering=False,
        debug=False,
        enable_asserts=False,
        num_devices=N_CORES,
    )
    # x ships bf16 for the same reason as the weights: the bf16 matmul
    # consumes bf16(x) either way; host-side cast == the casting DMA it
    # replaces, but rides the fast HWDGE path at half the wire bytes.
    xt = nc.dram_tensor("xt", [D, T], BF16, kind="ExternalInput").ap()
    # weights ship as bf16: identical rounding to an on-device f32->bf16 cast,
    # half the wire bytes, and staging stays on the fast HWDGE (non-casting)
    # DMA path. sign/|.| of the bf16 value match sign/|.| of the f32 value
    # to bf16 precision, which is all the bf16 matmul consumes anyway.
    w1t = nc.dram_tensor("w1t", [D, H], BF16, kind="ExternalInput").ap()
    b1 = nc.dram_tensor("b1", [H], F32, kind="ExternalInput").ap()
    w2t = nc.dram_tensor("w2t", [H, D], BF16, kind="ExternalInput").ap()
    b2 = nc.dram_tensor("b2", [D], F32, kind="ExternalInput").ap()
    out = nc.dram_tensor("out", [D, T], F32, kind="ExternalOutput").ap()

    w1t_3d = w1t.rearrange("(k p) h -> k p h", p=128)   # [KD, 128, H]
    w2t_3d = w2t.rearrange("(k p) d -> k p d", p=128)   # [KH, 128, D]
    xt_3d = xt.rearrange("(k p) t -> p k t", p=128)     # [128, KD, T]
    out_3d = out.rearrange("(m p) t -> m p t", p=128)   # [KD, 128, T]

    with tile.TileContext(nc) as tc:
        with (
            tc.tile_pool(name="wb", bufs=1) as wbpool,
            tc.tile_pool(name="consts", bufs=1) as cpool,
        ):
            # persistent binarized weights (bf16 +-1)
            w1b = wbpool.tile([128, KD, H], BF16, tag="w1b")
            w2b = wbpool.tile([128, KH, D], BF16, tag="w2b")
            # per-output-channel constants, partition-major per 128-tile
            alpha1c = cpool.tile([128, KH], F32, tag="a1")
            alpha2c = cpool.tile([128, KD], F32, tag="a2")
            b1c = cpool.tile([128, KH], F32, tag="b1")
            b2c = cpool.tile([128, KD], F32, tag="b2")
            ones1 = cpool.tile([128, 1], BF16, tag="ones1")
            ones2 = cpool.tile([128, 1], F32, tag="ones2")

            nc.vector.memset(ones1[:], 1.0 / D)
            nc.vector.memset(ones2[:], 1.0 / H)
            nc.sync.dma_start(b1c[:], b1.rearrange("(m p) -> p m", p=128))
            nc.sync.dma_start(b2c[:], b2.rearrange("(m p) -> p m", p=128))

            dpool = tc.alloc_tile_pool(name="adram", bufs=1, space="DRAM")
            a1d = dpool.tile([H], F32, tag="a1d")
            a2d = dpool.tile([D], F32, tag="a2d")

            # ---- w1 prep ----
            # alpha row = (1/D * ones)^T @ |w1t| with ones STATIONARY: the
            # 1-column LDWEIGHTS is free and the N=512 moving operand keeps the
            # PE array dense (N=1 matmuls starve the HAM activity monitor and
            # re-throttle the clock to 1.2GHz). The [1, H] row accumulates
            # per-bank (each 512-chunk is exactly one PSUM bank), then bounces
            # through DRAM to become the per-partition column layout.
            with (
                tc.tile_pool(name="w1stage", bufs=5) as s1pool,
                tc.tile_pool(name="absw1", bufs=2) as a1pool,
                tc.tile_pool(name="row1", bufs=1) as r1pool,
                tc.tile_pool(name="psrow1", bufs=1, space="PSUM") as pr1,
            ):
                with nc.named_scope("w1prep"):
                    a1row_ps = pr1.tile([1, H], F32, tag="a1row")
                    # prime the HAM clock gate: ~20 dense throwaway matmuls
                    # while the weight DMA streams in, so the 2.4GHz clock is
                    # already up when real work lands (results overwritten by
                    # the k==0 start=True below).
                    # warm rhs is ZEROS so keepalive matmuls can accumulate
                    # +0.0 into the live alpha row without corrupting it
                    warm = cpool.tile([128, 512], BF16, tag="warm")
                    nc.vector.memset(warm[:], 0.0)
                    for _ in range(20):
                        nc.tensor.matmul(
                            a1row_ps[:, 0:512],
                            lhsT=ones1[:],
                            rhs=warm[:],
                            start=False,
                            stop=False,
                            skip_group_check=True,
                        )
                    for k in range(KD):
                        if k > 0:
                            # HAM-keepalive filler: no-dep matmuls the PE can
                            # chew on while waiting for the next weight tile,
                            # so the clock gate never sees an idle window.
                            for _ in range(6):
                                nc.tensor.matmul(
                                    a1row_ps[:, 0:512],
                                    lhsT=ones1[:],
                                    rhs=warm[:],
                                    start=False,
                                    stop=False,
                                    skip_group_check=True,
                                )
                        st = s1pool.tile([128, H], BF16, tag="w1stage")
                        nc.sync.dma_start(st[:], w1t_3d[k])
                        # signs on ScalarE (idle until the first gelu), so the
                        # DVE queue is pure abs-ANDs and alpha1 lands early
                        nc.scalar.activation(w1b[:, k, :], st[:], AF.Sign)
                        ab = a1pool.tile([128, H], BF16, tag="absw1")
                        nc.vector.tensor_scalar(
                            ab[:].bitcast(mybir.dt.uint16),
                            st[:].bitcast(mybir.dt.uint16),
                            0x7FFF,
                            None,
                            ALU.bitwise_and,
                        )
                        for n in range(H // 512):
                            nc.tensor.matmul(
                                a1row_ps[:, ts(n, 512)],
                                lhsT=ones1[:],
                                rhs=ab[:, ts(n, 512)],
                                start=(k == 0),
                                stop=(k == KD - 1),
                                skip_group_check=True,
                            )
                    a1row = r1pool.tile([1, H], F32, tag="a1row_sb")
                    nc.scalar.copy(a1row[:], a1row_ps[:])
                    nc.sync.dma_start(a1d[:], a1row[:])
                    nc.sync.dma_start(
                        alpha1c[:], a1d.rearrange("(m p) -> p m", p=128)
                    )

            # ---- main: 4 token chunks, fc1 -> gelu -> fc2 ----
            with (
                tc.tile_pool(name="absw2", bufs=2) as a2pool,
                tc.tile_pool(name="row2", bufs=1) as r2pool,
                tc.tile_pool(name="xc", bufs=2) as xpool,
                tc.tile_pool(name="ht", bufs=1) as hpool,
                tc.tile_pool(name="oc", bufs=3) as opool,
                tc.tile_pool(name="psrow2", bufs=1, space="PSUM") as pr2,
                tc.tile_pool(name="ps1", bufs=4, space="PSUM") as ps1pool,
                tc.tile_pool(name="ps2", bufs=2, space="PSUM") as ps2pool,
            ):
                # w2 lands directly in its persistent SBUF tile (no staging
                # slots to recycle): DMA raw -> abs to scratch -> accumulate
                # sum|w2| on DVE (f32) -> sign in place. All of it hides
                # under fc1_c0; the partition reduce is then a single fp32
                # ones-stationary matmul pair (~1us of PE instead of a 14us
                # N=512 burst).
                a2row_ps = pr2.tile([1, D], F32, tag="a2row")
                acc2 = r2pool.tile([128, D], F32, tag="acc2")

                # chunk-0 x lands BEFORE the bulk w2 stream: it is on fc1's
                # critical path and must not queue behind 8MB of weights.
                xc0 = xpool.tile([128, KD, TC], BF16, tag="xc", name="xc0")
                nc.sync.dma_start(xc0[:], xt_3d[:, :, 0:TC])

                for k in range(KH):
                    nc.sync.dma_start(w2b[:, k, :], w2t_3d[k])
                for k in range(KH):
                    ab2 = a2pool.tile([128, D], BF16, tag="absw2",
                                      name=f"ab2_{k}")
                    nc.vector.tensor_scalar(
                        ab2[:].bitcast(mybir.dt.uint16),
                        w2b[:, k, :].bitcast(mybir.dt.uint16),
                        0x7FFF,
                        None,
                        ALU.bitwise_and,
                    )
                    # sign on DVE (ScalarE is loaded with gelus + w1 signs):
                    # bf16 sign(w) == (w & 0x8000) | 0x3f80, in place, ordered
                    # after the abs read by same-engine program order.
                    nc.vector.tensor_scalar(
                        w2b[:, k, :].bitcast(mybir.dt.uint16),
                        w2b[:, k, :].bitcast(mybir.dt.uint16),
                        0x8000,
                        0x3F80,
                        ALU.bitwise_and,
                        ALU.bitwise_or,
                    )
                    if k == 0:
                        nc.vector.tensor_copy(out=acc2[:], in_=ab2[:])
                    else:
                        nc.vector.tensor_add(
                            out=acc2[:], in0=acc2[:], in1=ab2[:]
                        )

                for c in range(N_CHUNK):
                    csl = slice(c * TC, (c + 1) * TC)
                    with nc.named_scope(f"fc1_c{c}"):
                        if c == 0:
                            xc = xc0
                        else:
                            xc = xpool.tile([128, KD, TC], BF16, tag="xc")
                            nc.sync.dma_start(xc[:], xt_3d[:, :, csl])
                        ht = hpool.tile([128, KH, TC], BF16, tag="ht")
                        for m in range(KH):
                            ps = ps1pool.tile([128, TC], F32, tag="ps1")
                            for k in range(KD):
                                nc.tensor.matmul(
                                    ps[:],
                                    lhsT=w1b[:, k, ts(m, 128)],
                                    rhs=xc[:, k, :],
                                    start=(k == 0),
                                    stop=(k == KD - 1),
                                )
                            nc.scalar.activation(
                                ht[:, m, :],
                                ps[:],
                                AF.Gelu,
                                bias=b1c[:, m : m + 1],
                                scale=alpha1c[:, m : m + 1],
                            )
                    if c == 0:
                        # alpha2 partition-reduce: one fp32 matmul pair over
                        # the DVE-accumulated sum|w2|, then the DRAM bounce
                        # into column layout.
                        with nc.named_scope("w2prep"):
                            for n in range(D // 512):
                                nc.tensor.matmul(
                                    a2row_ps[:, ts(n, 512)],
                                    lhsT=ones2[:],
                                    rhs=acc2[:, ts(n, 512)],
                                    start=True,
                                    stop=True,
                                    skip_group_check=True,
                                )
                            a2row = r2pool.tile([1, D], F32, tag="a2row_sb")
                            nc.scalar.copy(a2row[:], a2row_ps[:])
                            nc.sync.dma_start(a2d[:], a2row[:])
                            nc.sync.dma_start(
                                alpha2c[:], a2d.rearrange("(m p) -> p m", p=128)
                            )
                    with nc.named_scope(f"fc2_c{c}"):
                        for md in range(KD):
                            ps2 = ps2pool.tile([128, TC], F32, tag="ps2")
                            for mh in range(KH):
                                nc.tensor.matmul(
                                    ps2[:],
                                    lhsT=w2b[:, mh, ts(md, 128)],
                                    rhs=ht[:, mh, :],
                                    start=(mh == 0),
                                    stop=(mh == KH - 1),
                                )
                            oc = opool.tile([128, TC], F32, tag="oc")
                            nc.scalar.activation(
                                oc[:],
                                ps2[:],
                                AF.Identity,
                                bias=b2c[:, md : md + 1],
                                scale=alpha2c[:, md : md + 1],
                            )
                            nc.sync.dma_start(out_3d[md][:, csl], oc[:])

    nc.compile()
    return nc


_NC_CACHE = None


def _get_nc():
    global _NC_CACHE
    if _NC_CACHE is None:
        _NC_CACHE = build_kernel()
    return _NC_CACHE


def kernel(x, w1, b1, w2, b2):
    assert x.shape == (B, S, D) and w1.shape == (H, D) and w2.shape == (D, H)
    nc = _get_nc()

    import ml_dtypes

    xt = np.ascontiguousarray(x.reshape(T_GLOBAL, D).T).astype(
        ml_dtypes.bfloat16
    )                                                         # [D, 12544]
    w1t = np.ascontiguousarray(w1.T).astype(ml_dtypes.bfloat16)   # [D, H]
    w2t = np.ascontiguousarray(w2.T).astype(ml_dtypes.bfloat16)   # [H, D]
    b1 = np.ascontiguousarray(b1, dtype=np.float32)
    b2 = np.ascontiguousarray(b2, dtype=np.float32)

    in_maps = [
        {
            "xt": np.ascontiguousarray(xt[:, i * T : (i + 1) * T]),
            "w1t": w1t,
            "b1": b1,
            "w2t": w2t,
            "b2": b2,
        }
        for i in range(N_CORES)
    ]

    trace = bool(int(os.environ.get("BIMLP_TRACE", "0")))
    res = run_bass_kernel_spmd(
        nc, in_maps, core_ids=list(range(N_CORES)), trace=trace
    )
    if trace:
        kernel.last_results = res

    outt = np.concatenate([res.results[i]["out"] for i in range(N_CORES)], axis=1)
    return np.ascontiguousarray(outt.T).reshape(B, S, D).astype(np.float32)
